# revision 1
# baseline (speedup 1.0000x reference)
"""Trainium2 Bass kernel for an AttentionBlock (GroupNorm + MHA + proj + residual).

Shapes (hardcoded): x (16, 512, 32, 32) f32, 8 heads (ch=64), GN groups=32,
w_qkv (1536, 512), w_proj (512, 512).

Strategy: data-parallel over batch across 8 NeuronCores (2 batches/core, no
collectives). All large matmuls run in float32r (full PE rate for free dim
>= 256). Scores are computed transposed (s on partitions, t free) so the
softmax denominator falls out of the attn@v matmul as a 65th output row
(ones column appended to v^T); no cross-partition reductions anywhere.
GroupNorm statistics use DVE reductions + a tiny block-diagonal matmul, and
rsqrt is computed with a DVE-only Newton iteration (no ACT table swaps —
ScalarE does nothing but exp, which is the bottleneck engine).
"""
import numpy as np
from contextlib import ExitStack

import concourse.bass as bass
import concourse.mybir as mybir
import concourse.tile as tile
from concourse import bacc
from concourse.bass_utils import run_bass_kernel_spmd

F32 = mybir.dt.float32
F32R = mybir.dt.float32r
AF = mybir.ActivationFunctionType
OP = mybir.AluOpType

B, C, H, W = 16, 512, 32, 32
N = H * W            # 1024
NHEADS = 8
CH = C // NHEADS     # 64
NGROUPS = 32
GSIZE = C // NGROUPS  # 16 channels per group
EPS = 1e-5
NCORES = 8
BPC = B // NCORES    # batches per core = 2
NT = C // 128        # channel tiles per batch = 4
VW = NHEADS * (CH + 1)  # v_ext free width = 520

_cached = {}


def _build(dbg=False, reps=1, mm_bf16=False, interleave=False):
    MMDT = mybir.dt.bfloat16 if mm_bf16 else F32R
    dbuf = 2 if mm_bf16 else 1
    nc = bacc.Bacc("TRN2", target_bir_lowering=False, debug=False,
                   num_devices=NCORES)

    xd = nc.dram_tensor("x", [BPC, C, N], F32, kind="ExternalInput").ap()
    WDT = F32 if mm_bf16 else F32R
    wqk_d = nc.dram_tensor("wqk_t", [C, 2 * C], WDT, kind="ExternalInput").ap()
    wv_d = nc.dram_tensor("wv_ext", [C, VW], WDT, kind="ExternalInput").ap()
    wp_d = nc.dram_tensor("wp_t", [C, C], WDT, kind="ExternalInput").ap()
    bqk_d = nc.dram_tensor("bqk", [128, 8], F32, kind="ExternalInput").ap()
    bv_d = nc.dram_tensor("bv_bc", [128, VW], F32, kind="ExternalInput").ap()
    bp_d = nc.dram_tensor("bp", [128, NT], F32, kind="ExternalInput").ap()
    gam_d = nc.dram_tensor("gamma_t", [128, NT], F32, kind="ExternalInput").ap()
    bet_d = nc.dram_tensor("beta_t", [128, NT], F32, kind="ExternalInput").ap()
    bd_d = nc.dram_tensor("blockdiag16", [128, 8], F32, kind="ExternalInput").ap()
    bc_d = nc.dram_tensor("bcast16", [8, 128], F32, kind="ExternalInput").ap()
    outd = nc.dram_tensor("out", [BPC, C, N], F32, kind="ExternalOutput").ap()
    if dbg:
        dbg_xn = nc.dram_tensor("dbg_xn", [128, NT * N], F32R, kind="ExternalOutput").ap()
        dbg_qk = nc.dram_tensor("dbg_qk", [128, 8 * N], F32R, kind="ExternalOutput").ap()
        dbg_vv = nc.dram_tensor("dbg_vv", [128, 8 * VW], F32R, kind="ExternalOutput").ap()
        dbg_h = nc.dram_tensor("dbg_h", [128, NT * N], F32R, kind="ExternalOutput").ap()

    with tile.TileContext(nc) as tc, ExitStack() as ctx:
        # ---- pools ----
        wpool = ctx.enter_context(tc.tile_pool(name="weights", bufs=1))
        stage = ctx.enter_context(tc.tile_pool(name="stage", bufs=1)) if mm_bf16 else None
        xpool = ctx.enter_context(tc.tile_pool(name="x", bufs=2))
        xnpool = ctx.enter_context(tc.tile_pool(name="xn", bufs=dbuf))
        qkpool = ctx.enter_context(tc.tile_pool(name="qk", bufs=dbuf))
        vpool = ctx.enter_context(tc.tile_pool(name="v", bufs=dbuf))
        hpool = ctx.enter_context(tc.tile_pool(name="h", bufs=dbuf))
        ppool = ctx.enter_context(tc.tile_pool(name="p", bufs=5 if not mm_bf16 else 4))
        opool = ctx.enter_context(tc.tile_pool(name="o", bufs=4))
        small = ctx.enter_context(tc.tile_pool(name="small", bufs=2))
        scr = ctx.enter_context(tc.tile_pool(name="scr", bufs=1))
        ps_sc = ctx.enter_context(tc.tile_pool(name="ps_sc", bufs=2, space="PSUM"))
        ps_h = ctx.enter_context(tc.tile_pool(name="ps_h", bufs=2, space="PSUM"))

        # ---- load weights (f32r: direct DMA, the PE rounds operands
        #      identically to an explicit DVE rounding copy); load constants ----
        wqk_r, wv_r, wp_r = [], [], []
        for k in range(NT):
            wr = wpool.tile([128, 2 * C], MMDT, tag=f"wqk{k}")
            if mm_bf16:
                st = stage.tile([128, 2 * C], F32, tag="stage")
                nc.sync.dma_start(st[:], wqk_d[128 * k:128 * (k + 1), :])
                nc.vector.tensor_copy(wr[:], st[:])
            else:
                nc.sync.dma_start(wr[:], wqk_d[128 * k:128 * (k + 1), :])
            wqk_r.append(wr)
        for k in range(NT):
            wr = wpool.tile([128, VW], MMDT, tag=f"wv{k}")
            if mm_bf16:
                st = stage.tile([128, VW], F32, tag="stage")
                nc.sync.dma_start(st[:], wv_d[128 * k:128 * (k + 1), :])
                nc.vector.tensor_copy(wr[:], st[:])
            else:
                nc.sync.dma_start(wr[:], wv_d[128 * k:128 * (k + 1), :])
            wv_r.append(wr)
        for k in range(NT):
            wr = wpool.tile([128, C], MMDT, tag=f"wp{k}")
            if mm_bf16:
                st = stage.tile([128, C], F32, tag="stage")
                nc.sync.dma_start(st[:], wp_d[128 * k:128 * (k + 1), :])
                nc.vector.tensor_copy(wr[:], st[:])
            else:
                nc.sync.dma_start(wr[:], wp_d[128 * k:128 * (k + 1), :])
            wp_r.append(wr)

        bqk = wpool.tile([128, 8], F32, tag="bqk")
        nc.sync.dma_start(bqk[:], bqk_d[:])
        bv = wpool.tile([128, VW], F32, tag="bv")
        nc.sync.dma_start(bv[:], bv_d[:])
        bp = wpool.tile([128, NT], F32, tag="bp")
        nc.sync.dma_start(bp[:], bp_d[:])
        gam = wpool.tile([128, NT], F32, tag="gam")
        nc.sync.dma_start(gam[:], gam_d[:])
        bet = wpool.tile([128, NT], F32, tag="bet")
        nc.sync.dma_start(bet[:], bet_d[:])
        bd16 = wpool.tile([128, 8], F32, tag="bd16")
        nc.sync.dma_start(bd16[:], bd_d[:])
        bc16 = wpool.tile([8, 128], F32, tag="bc16")
        nc.sync.dma_start(bc16[:], bc_d[:])

        for b in [b for _ in range(reps) for b in range(BPC)]:
            # ---- load x ----
            x_sb = xpool.tile([128, NT * N], F32, tag="x")
            for j in range(NT):
                # ACT's HWDGE queue: SP's queue is busy with output stores at
                # batch boundaries, and ScalarE is idle during proj when these
                # fire, so the descriptor push doesn't cost exp time
                nc.scalar.dma_start(x_sb[:, N * j:N * (j + 1)],
                                    xd[b, 128 * j:128 * (j + 1), :])

            # ---- GroupNorm stats ----
            # stat cols 0..3 = per-chan-tile sums, 4..7 = sums of squares
            stat = small.tile([128, 8], F32, tag="stat")
            sq = scr.tile([128, N], F32, tag="sq")
            for j in range(NT):
                nc.vector.reduce_sum(stat[:, j:j + 1], x_sb[:, N * j:N * (j + 1)],
                                     axis=mybir.AxisListType.X)
                nc.vector.scalar_tensor_tensor(
                    sq[:], x_sb[:, N * j:N * (j + 1)], 1.0,
                    x_sb[:, N * j:N * (j + 1)],
                    op0=OP.bypass, op1=OP.mult,
                    accum_out=stat[:, 4 + j:5 + j])
            # group stats: (8 groups-per-tile, 8): [g, j] = sum, [g, 4+j] = sumsq
            ps_st = ps_sc.tile([8, 8], F32, tag="sc")
            nc.tensor.matmul(ps_st[:], bd16[:], stat[:], start=True, stop=True)
            inv = 1.0 / (GSIZE * N)
            mean8 = small.tile([8, 8], F32, tag="mean8")  # cols 0-3 mean, 4-7 rstd
            nc.vector.tensor_scalar_mul(mean8[:, 0:4], ps_st[:, 0:4], inv)
            ex2 = small.tile([8, 4], F32, tag="ex2")
            nc.vector.tensor_scalar_mul(ex2[:], ps_st[:, 4:8], inv)
            # veps = E[x^2] + eps - mean^2
            m2 = small.tile([8, 4], F32, tag="m2")
            nc.vector.tensor_mul(m2[:], mean8[:, 0:4], mean8[:, 0:4])
            veps = small.tile([8, 4], F32, tag="veps")
            nc.vector.scalar_tensor_tensor(veps[:], ex2[:], EPS, m2[:],
                                           op0=OP.add, op1=OP.subtract)
            # rstd = rsqrt(veps): r0 = 1.5 - 0.5 v, then 3 Newton steps
            r_cur = small.tile([8, 4], F32, tag="r0")
            nc.vector.tensor_scalar(r_cur[:], veps[:], -0.5, 1.5,
                                    op0=OP.mult, op1=OP.add)
            for it in range(3):
                t1 = small.tile([8, 4], F32, tag=f"nt1_{it}")
                nc.vector.tensor_mul(t1[:], r_cur[:], r_cur[:])
                t2 = small.tile([8, 4], F32, tag=f"nt2_{it}")
                nc.vector.scalar_tensor_tensor(t2[:], t1[:], -0.5, veps[:],
                                               op0=OP.mult, op1=OP.mult)
                t3 = small.tile([8, 4], F32, tag=f"nt3_{it}")
                nc.vector.tensor_scalar_add(t3[:], t2[:], 1.5)
                r_nxt = small.tile([8, 4], F32, tag=f"nr_{it}")
                nc.vector.tensor_mul(r_nxt[:], r_cur[:], t3[:])
                r_cur = r_nxt
            nc.vector.tensor_copy(mean8[:, 4:8], r_cur[:])
            # broadcast group stats to channels: (16,128)^T @ (8,8) -> (128,8)
            ps_bc = ps_sc.tile([128, 8], F32, tag="sc")
            nc.tensor.matmul(ps_bc[:], bc16[:], mean8[:], start=True, stop=True)
            # A = gamma * rstd_bcast ; Bb = beta - mean_bcast * A
            A_ch = small.tile([128, NT], F32, tag="A_ch")
            nc.vector.tensor_mul(A_ch[:], gam[:], ps_bc[:, 4:8])
            tB = small.tile([128, NT], F32, tag="tB")
            nc.vector.tensor_mul(tB[:], ps_bc[:, 0:4], A_ch[:])
            B_ch = small.tile([128, NT], F32, tag="B_ch")
            nc.vector.scalar_tensor_tensor(B_ch[:], tB[:], -1.0, bet[:],
                                           op0=OP.mult, op1=OP.add)
            # xn = A * x + B  (written as f32r for the matmuls)
            xn = xnpool.tile([128, NT * N], MMDT, tag="xn")
            for j in range(NT):
                nc.vector.tensor_scalar(xn[:, N * j:N * (j + 1)],
                                        x_sb[:, N * j:N * (j + 1)],
                                        A_ch[:, j:j + 1], B_ch[:, j:j + 1],
                                        op0=OP.mult, op1=OP.add)

            # ---- qkv projections + attention, pipelined ----
            # qk layout: cols 0..4095 = q (4 ch-tiles), 4096..8191 = k
            qk = qkpool.tile([128, 8 * N], MMDT, tag="qk")
            vv = vpool.tile([128, 8 * VW], MMDT, tag="vv")

            def emit_qk(o):
                for nh in range(2):
                    pq = (ps_sc if interleave else ps_h).tile([128, 512], F32, tag="sc" if interleave else "hacc", name=f"pq{o}_{nh}")
                    for k in range(NT):
                        nc.tensor.matmul(
                            pq[:],
                            wqk_r[k][:, 128 * o:128 * (o + 1)],
                            xn[:, N * k + 512 * nh:N * k + 512 * (nh + 1)],
                            start=(k == 0), stop=(k == NT - 1))
                    nc.vector.tensor_scalar_add(
                        qk[:, N * o + 512 * nh:N * o + 512 * (nh + 1)],
                        pq[:], bqk[:, o:o + 1])

            def emit_v(ntile):
                # v^T (+ones cols): out (128 n, 520) per n-tile
                for chh in range(2):
                    pv = (ps_sc if interleave else ps_h).tile([128, 260], F32, tag="sc" if interleave else "hacc", name=f"pv{ntile}_{chh}")
                    for k in range(NT):
                        nc.tensor.matmul(
                            pv[:],
                            xn[:, N * k + 128 * ntile:N * k + 128 * (ntile + 1)],
                            wv_r[k][:, 260 * chh:260 * (chh + 1)],
                            start=(k == 0), stop=(k == NT - 1))
                    nc.vector.tensor_add(
                        vv[:, VW * ntile + 260 * chh:VW * ntile + 260 * (chh + 1)],
                        pv[:], bv[:, 260 * chh:260 * (chh + 1)])

            if interleave:
                emit_qk(0)
                emit_qk(4)
                emit_v(0)
            else:
                for _o in range(8):
                    emit_qk(_o)
                for _nt in range(8):
                    emit_v(_nt)

            # ---- attention (head-sequential; even head = rows 0-63, odd head
            #      = rows 64-127 of q/k tile pr) ----
            hall = hpool.tile([128, NT * N], MMDT, tag="hall")

            def make_attnv(phs_, pr_):
                def attnv(side, j, p_tile):
                    head = 2 * pr_ + side
                    for th in range(2):
                        nc.tensor.matmul(
                            phs_[side][:, 512 * th:512 * (th + 1)],
                            vv[:, VW * j + (CH + 1) * head:
                               VW * j + (CH + 1) * head + CH + 1],
                            p_tile[:, 512 * th:512 * (th + 1)],
                            start=(j == 0), stop=(j == 7))
                return attnv

            def emit_tail(tail_):
                # previous pair's last attnv + h-normalize, emitted after the
                # next pair's first scores/exps: ScalarE starts the new pair
                # ~2us earlier, PSUM release order is unchanged
                attnv_, phs_, p_prev_, pr_ = tail_
                for side in range(2):
                    attnv_(side, 7, p_prev_[side])
                for side in range(2):
                    hc = scr.tile([65, N], F32, tag="hc", bufs=2)
                    nc.vector.tensor_copy(hc[:], phs_[side][:])
                    rec = small.tile([1, N], F32, tag="rec")
                    nc.vector.reciprocal(rec[:], hc[64:65, :])
                    rb = scr.tile([64, N], F32, tag="rb", bufs=2)
                    nc.gpsimd.partition_broadcast(rb[:], rec[:])
                    nc.vector.tensor_mul(
                        hall[64 * side:64 * side + 64, N * pr_:N * (pr_ + 1)],
                        hc[0:64, :], rb[:])

            tail = None
            for pr in range(4):
                q_base = N * pr
                k_base = 4 * N + N * pr
                phs = [ps_h.tile([65, N], F32, tag="hacc", name=f"phA{pr}"),
                       ps_h.tile([65, N], F32, tag="hacc", name=f"phB{pr}")]
                attnv = make_attnv(phs, pr)

                p_prev = [None, None]
                for j in range(8):
                    for side in range(2):
                        pb = 64 * side  # partition base within the qk tile
                        sc = ps_sc.tile([128, N], F32, tag="sc")
                        for th in range(2):
                            nc.tensor.matmul(
                                sc[:, 512 * th:512 * (th + 1)],
                                qk[pb:pb + 64, k_base + 128 * j:k_base + 128 * (j + 1)],
                                qk[pb:pb + 64, q_base + 512 * th:q_base + 512 * (th + 1)],
                                start=True, stop=True,
                                tile_position=(pb, 0))
                        p_t = ppool.tile([128, N], MMDT, tag="p")
                        nc.scalar.activation(p_t[:], sc[:], AF.Exp)
                        if p_prev[side] is not None:
                            attnv(side, j - 1, p_prev[side])
                        p_prev[side] = p_t
                        if j == 0 and side == 1 and tail is not None:
                            emit_tail(tail)
                            tail = None
                    if interleave and pr == 0 and j < 7:
                        emit_v(j + 1)
                tail = (attnv, phs, p_prev, pr)
                if interleave and pr < 3:
                    emit_qk(pr + 1)
                    emit_qk(4 + pr + 1)
            emit_tail(tail)

            if dbg and b == 0:
                nc.sync.dma_start(dbg_xn[:], xn[:])
                nc.sync.dma_start(dbg_qk[:], qk[:])
                nc.sync.dma_start(dbg_vv[:], vv[:])
                nc.sync.dma_start(dbg_h[:], hall[:])

            # ---- output projection + bias + residual ----
            for o in range(NT):
                for nh in range(2):
                    pp = ps_h.tile([128, 512], F32, tag="hacc")
                    for k in range(NT):
                        nc.tensor.matmul(
                            pp[:],
                            wp_r[k][:, 128 * o:128 * (o + 1)],
                            hall[:, N * k + 512 * nh:N * k + 512 * (nh + 1)],
                            start=(k == 0), stop=(k == NT - 1))
                    ot = opool.tile([128, 512], F32, tag="ot")
                    nc.vector.scalar_tensor_tensor(
                        ot[:], pp[:], bp[:, o:o + 1],
                        x_sb[:, N * o + 512 * nh:N * o + 512 * (nh + 1)],
                        op0=OP.add, op1=OP.add)
                    nc.sync.dma_start(
                        outd[b, 128 * o:128 * (o + 1), 512 * nh:512 * (nh + 1)],
                        ot[:])

    nc.compile()
    return nc


def _prep_shared(w_qkv, b_qkv, w_proj, b_proj, gamma, beta):
    qs = 1.0 / np.sqrt(np.sqrt(float(CH)))  # ch**-0.25
    s2 = qs * qs
    # reference maps qkv channel r -> head r//192, then q/k/v thirds within
    r = np.arange(3 * C).reshape(NHEADS, 3, CH)
    idx_q, idx_k, idx_v = r[:, 0].ravel(), r[:, 1].ravel(), r[:, 2].ravel()
    wqk_t = np.ascontiguousarray(
        np.concatenate([w_qkv[idx_q], w_qkv[idx_k]], axis=0).T).astype(np.float32)
    wqk_t[:, :C] *= s2
    bqk_full = np.concatenate([b_qkv[idx_q], b_qkv[idx_k]])
    bqk_full[:C] *= s2
    bqk = np.ascontiguousarray(bqk_full.reshape(8, 128).T).astype(np.float32)

    wv = w_qkv[idx_v]
    bv_src = b_qkv[idx_v]
    wv_ext = np.zeros((C, VW), np.float32)
    bv_ext = np.zeros((VW,), np.float32)
    for h in range(NHEADS):
        wv_ext[:, (CH + 1) * h:(CH + 1) * h + CH] = wv[CH * h:CH * (h + 1), :].T
        bv_ext[(CH + 1) * h:(CH + 1) * h + CH] = bv_src[CH * h:CH * (h + 1)]
        bv_ext[(CH + 1) * h + CH] = 1.0
    bv_bc = np.ascontiguousarray(np.broadcast_to(bv_ext, (128, VW))).astype(np.float32)

    wp_t = np.ascontiguousarray(w_proj.T).astype(np.float32)
    bp = np.ascontiguousarray(b_proj.reshape(NT, 128).T).astype(np.float32)
    gamma_t = np.ascontiguousarray(gamma.reshape(NT, 128).T).astype(np.float32)
    beta_t = np.ascontiguousarray(beta.reshape(NT, 128).T).astype(np.float32)
    blockdiag16 = np.kron(np.eye(8, dtype=np.float32), np.ones((GSIZE, 1), np.float32))
    bcast16 = np.ascontiguousarray(blockdiag16.T)
    return dict(wqk_t=wqk_t, bqk=bqk, wv_ext=wv_ext, bv_bc=bv_bc, wp_t=wp_t,
                bp=bp, gamma_t=gamma_t, beta_t=beta_t,
                blockdiag16=blockdiag16, bcast16=bcast16)


def kernel(x, gamma, beta, w_qkv, b_qkv, w_proj, b_proj):
    x = np.asarray(x, dtype=np.float32)
    shared = _prep_shared(np.asarray(w_qkv, np.float32), np.asarray(b_qkv, np.float32),
                          np.asarray(w_proj, np.float32), np.asarray(b_proj, np.float32),
                          np.asarray(gamma, np.float32), np.asarray(beta, np.float32))
    x6 = x.reshape(B, C, N)
    in_maps = [dict(x=np.ascontiguousarray(x6[BPC * i:BPC * (i + 1)]), **shared)
               for i in range(NCORES)]
    if "nc" not in _cached:
        _cached["nc"] = _build()
    res = run_bass_kernel_spmd(_cached["nc"], in_maps, list(range(NCORES)))
    out = np.empty((B, C, N), np.float32)
    for i in range(NCORES):
        out[BPC * i:BPC * (i + 1)] = res.results[i]["out"]
    return out.reshape(B, C, H, W)

